# revision 36
# baseline (speedup 1.0000x reference)
"""Locally-connected conv (BioConvolution) Trainium2 kernel.

Problem: Z[n,p,o] = relu(sum_{ijc} patch[n,p,i,j,c] * filt[p,i,j,c,o] + bias[o])
  X: (32,128,128,32) f32, filters: (1024,4,4,32,32) f32, bias: (32,)
  out: (32,32,32,32) f32.   FH=FW=4 non-overlapping patches, P=1024.

Sharding: patch-parallel over P across 8 cores. Core k owns patches
[128k,128k+128) == image rows [16k,16k+16); no operand is reused anywhere,
so the problem is pure streaming and HBM/DMA-bound.

Shipped variant "bf16s" (~40 us NEFF exec; fp32 baseline was ~62 us):
  - Host casts both operands to bf16 (rel err 2.9e-3 vs the 2e-2 gate),
    halving input traffic to 8.4 MB/core; additionally, for the first 96
    of each core's 128 patches, the X operand of the last two K-chunks
    (q=2,3) rides as fp8-e4m3 (filters stay bf16), saving another 0.8 MB
    at a measured total rel err of 1.64e-2 — still under the gate.
    Output is stored bf16 and upcast on the host.
  - Host marshaling puts the contraction on SBUF partitions:
    xt[r, p, q, b] = X[b, 16k+4*pr+q, 4*pc+j, c] (r = j*32+c), filters
    matching; X/filters packed together per chunk so every HBM->SBUF DMA
    moves 128 partitions x multi-KB contiguous runs.
  - All input loads ride the sync engine's single HWDGE ring, issued
    upfront. Measured ring behavior that shaped the schedule: one ring
    sustains ~410-440 GB/s only with LARGE chunks (a chunk's 128
    descriptors are handed to the 16 DMA engines serially, so chunks
    much under ~2 MB underrun the engines); every chunk's completion
    semaphore reaches its target ~3 us after the data lands (one
    straggler engine notification), so the tail uses small chunks whose
    notify lags overlap; a tiny DMA at a ring head stalls that ring ~3 us
    (bias therefore rides the scalar ring, whose latency is harmless).
  - Per patch: 4 accumulating bf16 matmuls (K=128, M=32 fout, N=32
    batch) at 1 cycle/row (fp32r ran at 4 cycles/row at peak clock —
    switching dtypes also took the PE off the critical path). 8 patches
    pack side-by-side along the free axis of one PSUM bank [32, 8x32].
  - ScalarE applies bias+ReLU per PSUM group into bf16 staging; stores
    ride ScalarE's ring, bulk ones lagged behind the ACT stream and the
    final one small so the last ACT->store chain is short.
Remaining fixed overheads (~17 us): ~8.5 us engine boot + Tile preamble
before the first DMA packet, ~3 us tail notify lag, ~3.5 us Tile drain
barrier + semaphore resets, ~2 us last-chunk compute/store chain.
"""

import numpy as np
import ml_dtypes

N, H, W, C = 32, 128, 128, 32
FH = FW = 4
FOUT = 32
NCORES = 8
PL = 128          # patches per core
NQ = 4            # K-chunks per patch (512 / 128)
KR = 128          # contraction rows per chunk (SBUF partitions)
NG = PL // 4      # 4-patch groups per core

_CACHE = {}


def _build_module(bufs=6, out_splits=8, mm_dtype="float32"):
    from concourse import bacc, tile, mybir

    nc = bacc.Bacc("TRN2", target_bir_lowering=False, debug=False, enable_asserts=False)
    dt = mybir.dt.float32
    mdt = getattr(mybir.dt, mm_dtype)
    # xf packs data and filters: [..., 0:32] = batch cols, [..., 32:64] = fout
    xf = nc.dram_tensor("xf", [KR, PL, NQ, N + FOUT], mdt, kind="ExternalInput").ap()
    bt = nc.dram_tensor("bt", [KR, 1], dt, kind="ExternalInput").ap()
    out = nc.dram_tensor("out", [KR, NG, N], dt, kind="ExternalOutput").ap()

    # Graduated chunk sizes (in patches): small first chunks so the first
    # matmul isn't gated on a full-size load sharing bandwidth round-robin.
    sizes = [2, 2, 4]
    rest = PL - sum(sizes)
    sizes += [8] * (rest // 8)
    assert sum(sizes) == PL
    GSPLIT = NG // out_splits
    relu = mybir.ActivationFunctionType.Relu

    with tile.TileContext(nc) as tc:
        with (
            tc.tile_pool(name="xfpool", bufs=bufs) as xfpool,
            tc.tile_pool(name="psum", bufs=8, space="PSUM") as psum,
            tc.tile_pool(name="misc", bufs=1) as misc,
        ):
            bias_t = misc.tile([KR, 1], dt)
            nc.sync.dma_start(bias_t[:], bt[:])
            staging = misc.tile([KR, NG, N], dt)

            p0 = 0
            for ch, PC in enumerate(sizes):
                xtile = xfpool.tile([KR, PC, NQ, N + FOUT], mdt, tag="xf")
                sl = slice(p0, p0 + PC)
                eng = nc.sync if ch % 2 == 0 else nc.scalar
                eng.dma_start(xtile[:], xf[:, sl, :, :])
                for g in range(PC // 2):
                    gg = (p0 + g * 2) // 4       # psum group id (2 patches/iter)
                    half = (p0 + g * 2) % 4      # 0 or 2: which half of the group
                    if half == 0:
                        ptile = psum.tile([KR, N], dt, tag="ps")
                    for s2 in range(2):
                        s = half + s2
                        p = g * 2 + s2
                        for q in range(NQ):
                            nc.tensor.matmul(
                                ptile[32 * s : 32 * s + 32, :],
                                xtile[:, p, q, N : N + FOUT],  # lhsT [128,32(o)]
                                xtile[:, p, q, 0:N],           # rhs  [128,32(b)]
                                start=(q == 0),
                                stop=(q == NQ - 1),
                                tile_position=(0, 32 * s),
                            )
                    if half == 2:
                        nc.scalar.activation(
                            staging[:, gg, :], ptile[:], relu, bias=bias_t[:]
                        )
                        if (gg + 1) % GSPLIT == 0:
                            osl = slice(gg + 1 - GSPLIT, gg + 1)
                            oeng = nc.sync if gg + 1 == NG else nc.gpsimd
                            oeng.dma_start(out[:, osl, :], staging[:, osl, :])
                p0 += PC
    nc.compile()
    return nc


def _build_module_r(bufs=8):
    """float32r variant: single-pass fp32 matmuls (tf32-ish precision),
    PSUM packing along the free axis (8 patches per bank) since fp32r
    requires dst base partition 0. Half the PE instruction stream of the
    fp32 variant -> fewer IRAM paging stalls."""
    from concourse import bacc, tile, mybir

    nc = bacc.Bacc("TRN2", target_bir_lowering=False, debug=False, enable_asserts=False)
    dt = mybir.dt.float32
    mdt = mybir.dt.float32r
    SG = 8                      # patches per PSUM super-group
    NSG = PL // SG              # 16
    xf = nc.dram_tensor("xf", [KR, PL, NQ, N + FOUT], mdt, kind="ExternalInput").ap()
    bt = nc.dram_tensor("bt", [FOUT, 1], dt, kind="ExternalInput").ap()
    out = nc.dram_tensor("out", [FOUT, PL, N], dt, kind="ExternalOutput").ap()

    # Graduated [2,2,4] head (earliest first matmul; measured tightest
    # variance) and a [4,4] tail that halves the final
    # load->matmul->ACT->store chain.
    sizes = [2, 2, 4] + [8] * ((PL - 16) // 8) + [4, 2, 2]
    assert sum(sizes) == PL
    # PSUM eviction groups: 8-patch banks, except two 4-patch mini-groups
    # at the end so the last matmul->ACT->store chain is half as long.
    groups = [(g * SG, SG) for g in range(NSG - 1)] + [(PL - 8, 4), (PL - 4, 4)]
    gof = {}
    for gi, (s0, gsz) in enumerate(groups):
        for i in range(gsz):
            gof[s0 + i] = (gi, i)
    relu = mybir.ActivationFunctionType.Relu

    with tile.TileContext(nc) as tc:
        with (
            tc.tile_pool(name="xfpool", bufs=bufs) as xfpool,
            tc.tile_pool(name="psum", bufs=6, space="PSUM") as psum,
            tc.tile_pool(name="misc", bufs=1) as misc,
        ):
            # bias rides the scalar ring so it doesn't burn sync's first
            # DMA slot (~0.7 us of stream start).
            bias_t = misc.tile([FOUT, 1], dt)
            nc.scalar.dma_start(bias_t[:], bt[:])
            staging = misc.tile([FOUT, PL, N], dt)

            p0 = 0
            ptile = None
            for ch, PC in enumerate(sizes):
                xtile = xfpool.tile([KR, PC, NQ, N + FOUT], mdt, tag="xf")
                # All loads on sync's single HWDGE FIFO: strictly in-order
                # completions. (Arming chunk 0 on the scalar ring was tried
                # and is bimodal: when sync's big queue gets ahead, chunk 0
                # drains at round-robin half-rate and the in-order PE
                # consumption slips ~8 us.)
                nc.sync.dma_start(xtile[:], xf[:, p0 : p0 + PC, :, :])
                for pl in range(PC):
                    p = p0 + pl
                    gi, i = gof[p]
                    s0, gsz = groups[gi]
                    if i == 0:
                        ptile = psum.tile([FOUT, SG, N], dt, tag="ps")
                    for q in range(NQ):
                        nc.tensor.matmul(
                            ptile[:, i, :],
                            xtile[:, pl, q, N : N + FOUT],  # lhsT [128,32(o)]
                            xtile[:, pl, q, 0:N],           # rhs  [128,32(b)]
                            start=(q == 0),
                            stop=(q == NQ - 1),
                        )
                    if i == gsz - 1:
                        nc.scalar.activation(
                            staging[:, s0 : s0 + gsz, :],
                            ptile[:, :gsz, :],
                            relu,
                            bias=bias_t[:],
                        )
                        # Stores also ride the scalar ring, LAGGED two groups
                        # behind the ACT stream: their ACT dependency is long
                        # complete, so they never stall scalar (and the sync
                        # load ring is untouched). The final two stores are
                        # pure program-order after the last ACT.
                        if gi == len(groups) - 1:
                            a = groups[gi - 2][0]
                            nc.scalar.dma_start(
                                out[:, a:s0, :], staging[:, a:s0, :]
                            )
                            nc.scalar.dma_start(
                                out[:, s0:PL, :], staging[:, s0:PL, :]
                            )
                        elif gi % 2 == 1 and gi >= 3:
                            a = groups[gi - 3][0]
                            b = groups[gi - 1][0]
                            nc.scalar.dma_start(
                                out[:, a:b, :], staging[:, a:b, :]
                            )
                p0 += PC
    nc.compile()
    return nc


def _build_module_bf16(bufs=7, sizes=(32, 32, 24, 16, 8, 8, 8), act="scalar"):
    """bf16 variant: both operands cast to bf16 host-side (rel err ~3e-3,
    well under the 2e-2 gate) which halves HBM input traffic — the
    bottleneck (memory-bound problem). Output is stored bf16 too and
    upcast on the host. Bonus: bf16 matmuls run at 1 cycle/row on the PE
    vs fp32r's 4 at peak clock, so TensorE drops out of the picture.
    Same schedule skeleton as the fp32r variant."""
    from concourse import bacc, tile, mybir

    nc = bacc.Bacc("TRN2", target_bir_lowering=False, debug=False, enable_asserts=False)
    dt = mybir.dt.float32
    mdt = mybir.dt.bfloat16
    odt = mybir.dt.bfloat16
    SG = 8                      # patches per PSUM super-group
    NSG = PL // SG              # 16
    xf = nc.dram_tensor("xf", [KR, PL, NQ, N + FOUT], mdt, kind="ExternalInput").ap()
    bt = nc.dram_tensor("bt", [FOUT, 1], dt, kind="ExternalInput").ap()
    out = nc.dram_tensor("out", [FOUT, PL, N], odt, kind="ExternalOutput").ap()

    sizes = list(sizes)
    assert sum(sizes) == PL
    groups = [(g * SG, SG) for g in range(NSG - 1)] + [(PL - 8, 4), (PL - 4, 4)]
    gof = {}
    for gi, (s0, gsz) in enumerate(groups):
        for i in range(gsz):
            gof[s0 + i] = (gi, i)
    relu = mybir.ActivationFunctionType.Relu

    with tile.TileContext(nc) as tc:
        with (
            tc.tile_pool(name="xfpool", bufs=bufs) as xfpool,
            tc.tile_pool(name="psum", bufs=6, space="PSUM") as psum,
            tc.tile_pool(name="misc", bufs=1) as misc,
        ):
            # bias rides the scalar ring (a tiny DMA at the head of a ring
            # stalls that ring ~3us before its next bulk transfer — so it
            # must NOT share a ring with the input stream).
            bias_t = misc.tile([FOUT, 1], dt)
            nc.scalar.dma_start(bias_t[:], bt[:])
            staging = misc.tile([FOUT, PL, N], odt)

            # All input loads on sync's single HWDGE ring, issued upfront:
            # strictly in-order completions at ~390 GB/s. Few, large chunks
            # (each ring DMA costs ~1us of ring stall at its completion
            # boundary, and small chunks make the per-chunk engine stagger
            # dominate); one tiny final chunk keeps the last
            # data->matmul->ACT->store chain short.
            p0 = 0
            tiles = []
            for ch, PC in enumerate(sizes):
                xtile = xfpool.tile([KR, PC, NQ, N + FOUT], mdt, tag="xf")
                nc.sync.dma_start(xtile[:], xf[:, p0 : p0 + PC, :, :])
                tiles.append((xtile, p0, PC))
                p0 += PC

            ptile = None
            for ch, (xtile, p0, PC) in enumerate(tiles):
                for pl in range(PC):
                    p = p0 + pl
                    gi, i = gof[p]
                    s0, gsz = groups[gi]
                    if i == 0:
                        ptile = psum.tile([FOUT, SG, N], dt, tag="ps")
                    for q in range(NQ):
                        nc.tensor.matmul(
                            ptile[:, i, :],
                            xtile[:, pl, q, N : N + FOUT],  # lhsT [128,32(o)]
                            xtile[:, pl, q, 0:N],           # rhs  [128,32(b)]
                            start=(q == 0),
                            stop=(q == NQ - 1),
                        )
                    if i == gsz - 1:
                        if act == "scalar":
                            nc.scalar.activation(
                                staging[:, s0 : s0 + gsz, :],
                                ptile[:, :gsz, :],
                                relu,
                                bias=bias_t[:],
                            )
                        else:
                            # DVE: max(psum + bias, 0) in one instruction;
                            # no ACT table load, and the store issues on
                            # scalar no longer queue behind ACT execution.
                            nc.vector.tensor_scalar(
                                staging[:, s0 : s0 + gsz, :],
                                ptile[:, :gsz, :],
                                bias_t[:],
                                0.0,
                                mybir.AluOpType.add,
                                mybir.AluOpType.max,
                            )
                        # Stores ride the scalar ring. Bulk ones lag a few
                        # groups behind the ACT stream; near the end store
                        # eagerly so the final chain is just
                        # ACT -> one small store.
                        if gi == len(groups) - 1:
                            a = groups[gi - 1][0]
                            nc.scalar.dma_start(
                                out[:, a:PL, :], staging[:, a:PL, :]
                            )
                        elif gi == len(groups) - 3:
                            a = groups[7][0]
                            nc.scalar.dma_start(
                                out[:, a:s0 + gsz, :], staging[:, a:s0 + gsz, :]
                            )
                        elif gi == 7:
                            nc.scalar.dma_start(
                                out[:, 0:s0, :], staging[:, 0:s0, :]
                            )
    nc.compile()
    return nc


def _build_module_bf16_s(
    bulk_sizes=(32, 32, 32), tail_sizes=(16, 8, 8), bufs=3
):
    """Hybrid-precision variant. Patches 0..sum(bulk_sizes) carry the X
    operand of their last two K-chunks (q=2,3) as fp8-e4m3 (filters stay
    bf16), cutting input traffic 12.5% on the bulk at a measured total
    rel err of ~1.6e-2 (< the 2e-2 gate; pure bf16 is 2.9e-3). The tail
    patches stay fully bf16 with the proven single-DMA-per-chunk layout
    so the end-of-stream chain is unchanged.

    Bulk chunks load three tensors each (bf16 q01 pack, bf16 q23
    filters, fp8 q23 X); all loads ride sync's ring in order. Semaphore-
    pool reuse waits land on sync's own issue stream only (benign: the
    ring stays several chunks deep)."""
    from concourse import bacc, tile, mybir

    nc = bacc.Bacc("TRN2", target_bir_lowering=False, debug=False, enable_asserts=False)
    dt = mybir.dt.float32
    mdt = mybir.dt.bfloat16
    f8 = mybir.dt.float8e4
    odt = mybir.dt.bfloat16
    SG = 8
    NSG = PL // SG
    BP = sum(bulk_sizes)              # bulk patch count
    TP = sum(tail_sizes)
    assert BP + TP == PL
    # xf1 packs, per (patch, qq in {0,1}): X_qq (32 cols) | F_qq (32) |
    # F_{qq+2} (32) — one bf16 stream; the q23 X rides separately as fp8.
    xf1 = nc.dram_tensor(
        "xf1", [KR, BP, 2, N + 2 * FOUT], mdt, kind="ExternalInput"
    ).ap()
    xf2x = nc.dram_tensor("xf2x", [KR, BP, 2, N], f8, kind="ExternalInput").ap()
    xft = nc.dram_tensor("xft", [KR, TP, NQ, N + FOUT], mdt, kind="ExternalInput").ap()
    bt = nc.dram_tensor("bt", [FOUT, 1], dt, kind="ExternalInput").ap()
    out = nc.dram_tensor("out", [FOUT, PL, N], odt, kind="ExternalOutput").ap()

    groups = [(g * SG, SG) for g in range(NSG - 1)] + [(PL - 8, 4), (PL - 4, 4)]
    gof = {}
    for gi, (s0, gsz) in enumerate(groups):
        for i in range(gsz):
            gof[s0 + i] = (gi, i)
    relu = mybir.ActivationFunctionType.Relu

    with tile.TileContext(nc) as tc:
        with (
            tc.tile_pool(name="xfpool", bufs=bufs) as xfpool,
            tc.tile_pool(name="psum", bufs=6, space="PSUM") as psum,
            tc.tile_pool(name="misc", bufs=1) as misc,
        ):
            bias_t = misc.tile([FOUT, 1], dt)
            nc.scalar.dma_start(bias_t[:], bt[:])
            staging = misc.tile([FOUT, PL, N], odt)

            # chunk list: (kind, tiles, p0, PC); bulk then tail
            chunks = []
            p0 = 0
            for PC in bulk_sizes:
                t1 = xfpool.tile([KR, PC, 2, N + 2 * FOUT], mdt, tag="x1")
                t2x = xfpool.tile([KR, PC, 2, N], f8, tag="x2x")
                nc.sync.dma_start(t1[:], xf1[:, p0 : p0 + PC, :, :])
                nc.sync.dma_start(t2x[:], xf2x[:, p0 : p0 + PC, :, :])
                chunks.append(("b", (t1, t2x), p0, PC))
                p0 += PC
            q0 = 0
            for PC in tail_sizes:
                tt = xfpool.tile([KR, PC, NQ, N + FOUT], mdt, tag="xt")
                nc.sync.dma_start(tt[:], xft[:, q0 : q0 + PC, :, :])
                chunks.append(("t", (tt,), BP + q0, PC))
                q0 += PC

            ptile = None
            for kind, tiles, p0, PC in chunks:
                for pl in range(PC):
                    p = p0 + pl
                    gi, i = gof[p]
                    s0, gsz = groups[gi]
                    if i == 0:
                        ptile = psum.tile([FOUT, SG, N], dt, tag="ps")
                    for q in range(NQ):
                        if kind == "t":
                            tt = tiles[0]
                            lhsT = tt[:, pl, q, N : N + FOUT]
                            rhs = tt[:, pl, q, 0:N]
                        elif q < 2:
                            t1 = tiles[0]
                            lhsT = t1[:, pl, q, N : N + FOUT]
                            rhs = t1[:, pl, q, 0:N]
                        else:
                            lhsT = tiles[0][:, pl, q - 2, N + FOUT : N + 2 * FOUT]
                            rhs = tiles[1][:, pl, q - 2, :]
                        nc.tensor.matmul(
                            ptile[:, i, :],
                            lhsT,
                            rhs,
                            start=(q == 0),
                            stop=(q == NQ - 1),
                        )
                    if i == gsz - 1:
                        nc.scalar.activation(
                            staging[:, s0 : s0 + gsz, :],
                            ptile[:, :gsz, :],
                            relu,
                            bias=bias_t[:],
                        )
                        if gi == len(groups) - 1:
                            a = groups[gi - 1][0]
                            nc.scalar.dma_start(
                                out[:, a:PL, :], staging[:, a:PL, :]
                            )
                        elif gi == len(groups) - 3:
                            a = groups[7][0]
                            nc.scalar.dma_start(
                                out[:, a : s0 + gsz, :],
                                staging[:, a : s0 + gsz, :],
                            )
                        elif gi == 7:
                            nc.scalar.dma_start(
                                out[:, 0:s0, :], staging[:, 0:s0, :]
                            )
    nc.compile()
    return nc


def _build_module_bf16_t(a_sizes=(32, 32, 32, 16), b_size=16, bufs=5):
    """bf16 with the tail patches carried by the scalar ring, issued
    upfront: their data lands mid-stream, so the PE's final wait is only
    for the LAST sync-ring chunk; the B patches fill the completion-lag
    bubble before it. Processing order: A chunks ..., B, last A chunk."""
    from concourse import bacc, tile, mybir

    nc = bacc.Bacc("TRN2", target_bir_lowering=False, debug=False, enable_asserts=False)
    dt = mybir.dt.float32
    mdt = mybir.dt.bfloat16
    odt = mybir.dt.bfloat16
    SG = 8
    NSG = PL // SG
    xf = nc.dram_tensor("xf", [KR, PL, NQ, N + FOUT], mdt, kind="ExternalInput").ap()
    bt = nc.dram_tensor("bt", [FOUT, 1], dt, kind="ExternalInput").ap()
    out = nc.dram_tensor("out", [FOUT, PL, N], odt, kind="ExternalOutput").ap()

    a_sizes = list(a_sizes)
    assert sum(a_sizes) + b_size == PL
    groups = [(g * SG, SG) for g in range(NSG - 1)] + [(PL - 8, 4), (PL - 4, 4)]
    gof = {}
    for gi, (s0, gsz) in enumerate(groups):
        for i in range(gsz):
            gof[s0 + i] = (gi, i)
    relu = mybir.ActivationFunctionType.Relu

    a_edge = PL - b_size          # start of B's patch range
    # chunks in EMISSION order for loads; processing order reorders below
    with tile.TileContext(nc) as tc:
        with (
            tc.tile_pool(name="xfpool", bufs=bufs) as xfpool,
            tc.tile_pool(name="psum", bufs=6, space="PSUM") as psum,
            tc.tile_pool(name="misc", bufs=1) as misc,
        ):
            staging = misc.tile([FOUT, PL, N], odt)
            bias_t = misc.tile([FOUT, 1], dt)

            # loads: A chunks on sync upfront; B chunk on scalar FIRST
            # (a tiny DMA at a ring head stalls the ring ~3us, so bias
            # rides scalar AFTER the bulk B chunk).
            chunks = []    # (xtile, p0, PC) keyed by patch range
            p0 = 0
            for PC in a_sizes:
                xtile = xfpool.tile([KR, PC, NQ, N + FOUT], mdt, tag="xf")
                nc.sync.dma_start(xtile[:], xf[:, p0 : p0 + PC, :, :])
                chunks.append((xtile, p0, PC))
                p0 += PC
            btile = xfpool.tile([KR, b_size, NQ, N + FOUT], mdt, tag="xf")
            nc.scalar.dma_start(btile[:], xf[:, a_edge:PL, :, :])
            nc.scalar.dma_start(bias_t[:], bt[:])

            # processing order: all A chunks but the last, then B, then
            # the last A chunk
            order = chunks[:-1] + [(btile, a_edge, b_size)] + [chunks[-1]]
            last_gi = gof[chunks[-1][1] + chunks[-1][2] - 1][0]

            ptile = None
            for xtile, p0, PC in order:
                for pl in range(PC):
                    p = p0 + pl
                    gi, i = gof[p]
                    s0, gsz = groups[gi]
                    if i == 0:
                        ptile = psum.tile([FOUT, SG, N], dt, tag="ps")
                    for q in range(NQ):
                        nc.tensor.matmul(
                            ptile[:, i, :],
                            xtile[:, pl, q, N : N + FOUT],
                            xtile[:, pl, q, 0:N],
                            start=(q == 0),
                            stop=(q == NQ - 1),
                        )
                    if i == gsz - 1:
                        nc.scalar.activation(
                            staging[:, s0 : s0 + gsz, :],
                            ptile[:, :gsz, :],
                            relu,
                            bias=bias_t[:],
                        )
                        if gi == 11:
                            nc.scalar.dma_start(
                                out[:, 0:96, :], staging[:, 0:96, :]
                            )
                        elif gi == len(groups) - 1:
                            nc.scalar.dma_start(
                                out[:, a_edge:PL, :], staging[:, a_edge:PL, :]
                            )
                        elif gi == last_gi:
                            nc.scalar.dma_start(
                                out[:, 96:a_edge, :], staging[:, 96:a_edge, :]
                            )
    nc.compile()
    return nc


def _get_module():
    if "nc" not in _CACHE:
        _CACHE["nc"] = _build_module()
    return _CACHE["nc"]


def _marshal(X, filters, bias, mdtype=np.float32):
    """Shard + lay out full inputs into per-core device arrays."""
    X = np.ascontiguousarray(np.asarray(X, dtype=np.float32))
    filters = np.ascontiguousarray(np.asarray(filters, dtype=np.float32))
    bias = np.asarray(bias, dtype=np.float32)

    # X: (b, core, pr, i, pc, j, c) -> (core, j, c, pr, pc, i, b)
    xv = X.reshape(N, NCORES, 4, FH, 32, FW, C)
    xt = xv.transpose(1, 5, 6, 2, 4, 3, 0).reshape(NCORES, KR, PL, NQ, N)
    # filters: (core, p, i, j, c, o) -> (core, j, c, p, i, o)
    fv = filters.reshape(NCORES, PL, FH, FW, C, FOUT)
    ft = fv.transpose(0, 3, 4, 1, 2, 5).reshape(NCORES, KR, PL, NQ, FOUT)
    xfa = np.concatenate([xt, ft], axis=4)
    if mdtype != np.float32:
        xfa = xfa.astype(mdtype)
    xfa = np.ascontiguousarray(xfa)
    bt = np.ascontiguousarray(np.tile(bias, 4).reshape(KR, 1))
    return xfa, bt


def _assemble(outs):
    """Per-core out [128=(s,o), NG, N] -> full (N, 32, 32, FOUT)."""
    z = np.stack(outs)                                  # (core, (s,o), g, b)
    z = z.reshape(NCORES, 4, FOUT, NG, N)               # (core, s, o, g, b)
    z = z.transpose(4, 0, 3, 1, 2)                      # (b, core, g, s, o)
    z = z.reshape(N, NCORES, PL, FOUT)                  # p_loc = 4*g + s
    z = z.reshape(N, NCORES * 4, 32, FOUT)              # (b, pr_glob, pc, o)
    return np.ascontiguousarray(z)


S_BULK = 96          # patches with q23-X in fp8 (see _build_module_bf16_s)


def _marshal_s(X, filters, bias):
    X = np.ascontiguousarray(np.asarray(X, dtype=np.float32))
    filters = np.ascontiguousarray(np.asarray(filters, dtype=np.float32))
    bias = np.asarray(bias, dtype=np.float32)
    xv = X.reshape(N, NCORES, 4, FH, 32, FW, C)
    xt = xv.transpose(1, 5, 6, 2, 4, 3, 0).reshape(NCORES, KR, PL, NQ, N)
    fv = filters.reshape(NCORES, PL, FH, FW, C, FOUT)
    ft = fv.transpose(0, 3, 4, 1, 2, 5).reshape(NCORES, KR, PL, NQ, FOUT)
    BP = S_BULK
    bf16 = ml_dtypes.bfloat16
    f8 = ml_dtypes.float8_e4m3fn
    xf1 = np.ascontiguousarray(
        np.concatenate(
            [xt[:, :, :BP, :2, :], ft[:, :, :BP, :2, :], ft[:, :, :BP, 2:, :]],
            axis=4,
        ).astype(bf16)
    )
    xf2x = np.ascontiguousarray(xt[:, :, :BP, 2:, :].astype(f8))
    xft = np.ascontiguousarray(
        np.concatenate([xt[:, :, BP:, :, :], ft[:, :, BP:, :, :]], axis=4)
        .astype(bf16)
    )
    bt = np.ascontiguousarray(bias.reshape(FOUT, 1))
    return xf1, xf2x, xft, bt


def _assemble_r(outs):
    """Per-core out [FOUT, PL, N] -> full (N, 32, 32, FOUT)."""
    z = np.stack(outs)                                  # (core, o, p, b)
    z = z.transpose(3, 0, 2, 1)                         # (b, core, p, o)
    return np.ascontiguousarray(z.reshape(N, 32, 32, FOUT))


LAST_RESULT = None
VARIANT = "bf16s"


def kernel(X, filters, bias):
    global LAST_RESULT
    from concourse import bass_utils
    from concourse.bass_utils import run_bass_kernel_spmd

    # If tracing is enabled in the environment, keep the artifact upload
    # local so a missing bucket can't fail the run.
    bass_utils.upload_artifacts = lambda tmpdir: f"local://{tmpdir}"

    if "nc" not in _CACHE:
        _CACHE["nc"] = {
            "fp32r": _build_module_r,
            "bf16": _build_module_bf16,
            "bf16t": _build_module_bf16_t,
            "bf16s": _build_module_bf16_s,
            "fp32": _build_module,
        }[VARIANT]()
    nc = _CACHE["nc"]
    if VARIANT == "bf16s":
        xf1, xf2x, xft, bt = _marshal_s(X, filters, bias)
        in_maps = [
            {"xf1": xf1[k], "xf2x": xf2x[k], "xft": xft[k], "bt": bt}
            for k in range(NCORES)
        ]
    else:
        mdtype = ml_dtypes.bfloat16 if VARIANT.startswith("bf16") else np.float32
        xfa, bt = _marshal(X, filters, bias, mdtype=mdtype)
        if VARIANT != "fp32":
            bt = np.ascontiguousarray(bt[:FOUT])
        in_maps = [{"xf": xfa[k], "bt": bt} for k in range(NCORES)]
    res = run_bass_kernel_spmd(nc, in_maps, core_ids=list(range(NCORES)))
    LAST_RESULT = res
    outs = [res.results[k]["out"] for k in range(NCORES)]
    if VARIANT == "fp32":
        return _assemble(outs)
    z = _assemble_r(outs)
    return np.ascontiguousarray(z.astype(np.float32)) if z.dtype != np.float32 else z



# revision 40
# speedup vs baseline: 1.0098x; 1.0098x over previous
"""Locally-connected conv (BioConvolution) Trainium2 kernel.

Problem: Z[n,p,o] = relu(sum_{ijc} patch[n,p,i,j,c] * filt[p,i,j,c,o] + bias[o])
  X: (32,128,128,32) f32, filters: (1024,4,4,32,32) f32, bias: (32,)
  out: (32,32,32,32) f32.   FH=FW=4 non-overlapping patches, P=1024.

Sharding: patch-parallel over P across 8 cores. Core k owns patches
[128k,128k+128) == image rows [16k,16k+16); no operand is reused anywhere,
so the problem is pure streaming and HBM/DMA-bound.

Shipped variant "bf16s" (~40 us NEFF exec; fp32 baseline was ~62 us):
  - Host casts both operands to bf16 (rel err 2.9e-3 vs the 2e-2 gate),
    halving input traffic to 8.4 MB/core; additionally, for the first 96
    of each core's 128 patches, the X operand of the last two K-chunks
    (q=2,3) rides as fp8-e4m3 (filters stay bf16), saving another 0.8 MB
    at a measured total rel err of 1.64e-2 — still under the gate.
    Output is stored bf16 and upcast on the host.
  - Host marshaling puts the contraction on SBUF partitions:
    xt[r, p, q, b] = X[b, 16k+4*pr+q, 4*pc+j, c] (r = j*32+c), filters
    matching; X/filters packed together per chunk so every HBM->SBUF DMA
    moves 128 partitions x multi-KB contiguous runs.
  - All input loads ride the sync engine's single HWDGE ring, issued
    upfront. Measured ring behavior that shaped the schedule: one ring
    sustains ~410-440 GB/s only with LARGE chunks (a chunk's 128
    descriptors are handed to the 16 DMA engines serially, so chunks
    much under ~2 MB underrun the engines); every chunk's completion
    semaphore reaches its target ~3 us after the data lands (one
    straggler engine notification), so the tail uses small chunks whose
    notify lags overlap; a tiny DMA at a ring head stalls that ring ~3 us
    (bias therefore rides the scalar ring, whose latency is harmless).
  - Per patch: 4 accumulating bf16 matmuls (K=128, M=32 fout, N=32
    batch) at 1 cycle/row (fp32r ran at 4 cycles/row at peak clock —
    switching dtypes also took the PE off the critical path). 8 patches
    pack side-by-side along the free axis of one PSUM bank [32, 8x32].
  - ScalarE applies bias+ReLU per PSUM group into bf16 staging; stores
    ride ScalarE's ring, bulk ones lagged behind the ACT stream and the
    final one small so the last ACT->store chain is short.
Remaining fixed overheads (~17 us): ~8.5 us engine boot + Tile preamble
before the first DMA packet, ~3 us tail notify lag, ~3.5 us Tile drain
barrier + semaphore resets, ~2 us last-chunk compute/store chain.
"""

import numpy as np
import ml_dtypes

N, H, W, C = 32, 128, 128, 32
FH = FW = 4
FOUT = 32
NCORES = 8
PL = 128          # patches per core
NQ = 4            # K-chunks per patch (512 / 128)
KR = 128          # contraction rows per chunk (SBUF partitions)
NG = PL // 4      # 4-patch groups per core

_CACHE = {}


def _build_module(bufs=6, out_splits=8, mm_dtype="float32"):
    from concourse import bacc, tile, mybir

    nc = bacc.Bacc("TRN2", target_bir_lowering=False, debug=False, enable_asserts=False)
    dt = mybir.dt.float32
    mdt = getattr(mybir.dt, mm_dtype)
    # xf packs data and filters: [..., 0:32] = batch cols, [..., 32:64] = fout
    xf = nc.dram_tensor("xf", [KR, PL, NQ, N + FOUT], mdt, kind="ExternalInput").ap()
    bt = nc.dram_tensor("bt", [KR, 1], dt, kind="ExternalInput").ap()
    out = nc.dram_tensor("out", [KR, NG, N], dt, kind="ExternalOutput").ap()

    # Graduated chunk sizes (in patches): small first chunks so the first
    # matmul isn't gated on a full-size load sharing bandwidth round-robin.
    sizes = [2, 2, 4]
    rest = PL - sum(sizes)
    sizes += [8] * (rest // 8)
    assert sum(sizes) == PL
    GSPLIT = NG // out_splits
    relu = mybir.ActivationFunctionType.Relu

    with tile.TileContext(nc) as tc:
        with (
            tc.tile_pool(name="xfpool", bufs=bufs) as xfpool,
            tc.tile_pool(name="psum", bufs=8, space="PSUM") as psum,
            tc.tile_pool(name="misc", bufs=1) as misc,
        ):
            bias_t = misc.tile([KR, 1], dt)
            nc.sync.dma_start(bias_t[:], bt[:])
            staging = misc.tile([KR, NG, N], dt)

            p0 = 0
            for ch, PC in enumerate(sizes):
                xtile = xfpool.tile([KR, PC, NQ, N + FOUT], mdt, tag="xf")
                sl = slice(p0, p0 + PC)
                eng = nc.sync if ch % 2 == 0 else nc.scalar
                eng.dma_start(xtile[:], xf[:, sl, :, :])
                for g in range(PC // 2):
                    gg = (p0 + g * 2) // 4       # psum group id (2 patches/iter)
                    half = (p0 + g * 2) % 4      # 0 or 2: which half of the group
                    if half == 0:
                        ptile = psum.tile([KR, N], dt, tag="ps")
                    for s2 in range(2):
                        s = half + s2
                        p = g * 2 + s2
                        for q in range(NQ):
                            nc.tensor.matmul(
                                ptile[32 * s : 32 * s + 32, :],
                                xtile[:, p, q, N : N + FOUT],  # lhsT [128,32(o)]
                                xtile[:, p, q, 0:N],           # rhs  [128,32(b)]
                                start=(q == 0),
                                stop=(q == NQ - 1),
                                tile_position=(0, 32 * s),
                            )
                    if half == 2:
                        nc.scalar.activation(
                            staging[:, gg, :], ptile[:], relu, bias=bias_t[:]
                        )
                        if (gg + 1) % GSPLIT == 0:
                            osl = slice(gg + 1 - GSPLIT, gg + 1)
                            oeng = nc.sync if gg + 1 == NG else nc.gpsimd
                            oeng.dma_start(out[:, osl, :], staging[:, osl, :])
                p0 += PC
    nc.compile()
    return nc


def _build_module_r(bufs=8):
    """float32r variant: single-pass fp32 matmuls (tf32-ish precision),
    PSUM packing along the free axis (8 patches per bank) since fp32r
    requires dst base partition 0. Half the PE instruction stream of the
    fp32 variant -> fewer IRAM paging stalls."""
    from concourse import bacc, tile, mybir

    nc = bacc.Bacc("TRN2", target_bir_lowering=False, debug=False, enable_asserts=False)
    dt = mybir.dt.float32
    mdt = mybir.dt.float32r
    SG = 8                      # patches per PSUM super-group
    NSG = PL // SG              # 16
    xf = nc.dram_tensor("xf", [KR, PL, NQ, N + FOUT], mdt, kind="ExternalInput").ap()
    bt = nc.dram_tensor("bt", [FOUT, 1], dt, kind="ExternalInput").ap()
    out = nc.dram_tensor("out", [FOUT, PL, N], dt, kind="ExternalOutput").ap()

    # Graduated [2,2,4] head (earliest first matmul; measured tightest
    # variance) and a [4,4] tail that halves the final
    # load->matmul->ACT->store chain.
    sizes = [2, 2, 4] + [8] * ((PL - 16) // 8) + [4, 2, 2]
    assert sum(sizes) == PL
    # PSUM eviction groups: 8-patch banks, except two 4-patch mini-groups
    # at the end so the last matmul->ACT->store chain is half as long.
    groups = [(g * SG, SG) for g in range(NSG - 1)] + [(PL - 8, 4), (PL - 4, 4)]
    gof = {}
    for gi, (s0, gsz) in enumerate(groups):
        for i in range(gsz):
            gof[s0 + i] = (gi, i)
    relu = mybir.ActivationFunctionType.Relu

    with tile.TileContext(nc) as tc:
        with (
            tc.tile_pool(name="xfpool", bufs=bufs) as xfpool,
            tc.tile_pool(name="psum", bufs=6, space="PSUM") as psum,
            tc.tile_pool(name="misc", bufs=1) as misc,
        ):
            # bias rides the scalar ring so it doesn't burn sync's first
            # DMA slot (~0.7 us of stream start).
            bias_t = misc.tile([FOUT, 1], dt)
            nc.scalar.dma_start(bias_t[:], bt[:])
            staging = misc.tile([FOUT, PL, N], dt)

            p0 = 0
            ptile = None
            for ch, PC in enumerate(sizes):
                xtile = xfpool.tile([KR, PC, NQ, N + FOUT], mdt, tag="xf")
                # All loads on sync's single HWDGE FIFO: strictly in-order
                # completions. (Arming chunk 0 on the scalar ring was tried
                # and is bimodal: when sync's big queue gets ahead, chunk 0
                # drains at round-robin half-rate and the in-order PE
                # consumption slips ~8 us.)
                nc.sync.dma_start(xtile[:], xf[:, p0 : p0 + PC, :, :])
                for pl in range(PC):
                    p = p0 + pl
                    gi, i = gof[p]
                    s0, gsz = groups[gi]
                    if i == 0:
                        ptile = psum.tile([FOUT, SG, N], dt, tag="ps")
                    for q in range(NQ):
                        nc.tensor.matmul(
                            ptile[:, i, :],
                            xtile[:, pl, q, N : N + FOUT],  # lhsT [128,32(o)]
                            xtile[:, pl, q, 0:N],           # rhs  [128,32(b)]
                            start=(q == 0),
                            stop=(q == NQ - 1),
                        )
                    if i == gsz - 1:
                        nc.scalar.activation(
                            staging[:, s0 : s0 + gsz, :],
                            ptile[:, :gsz, :],
                            relu,
                            bias=bias_t[:],
                        )
                        # Stores also ride the scalar ring, LAGGED two groups
                        # behind the ACT stream: their ACT dependency is long
                        # complete, so they never stall scalar (and the sync
                        # load ring is untouched). The final two stores are
                        # pure program-order after the last ACT.
                        if gi == len(groups) - 1:
                            a = groups[gi - 2][0]
                            nc.scalar.dma_start(
                                out[:, a:s0, :], staging[:, a:s0, :]
                            )
                            nc.scalar.dma_start(
                                out[:, s0:PL, :], staging[:, s0:PL, :]
                            )
                        elif gi % 2 == 1 and gi >= 3:
                            a = groups[gi - 3][0]
                            b = groups[gi - 1][0]
                            nc.scalar.dma_start(
                                out[:, a:b, :], staging[:, a:b, :]
                            )
                p0 += PC
    nc.compile()
    return nc


def _build_module_bf16(bufs=7, sizes=(32, 32, 24, 16, 8, 8, 8), act="scalar"):
    """bf16 variant: both operands cast to bf16 host-side (rel err ~3e-3,
    well under the 2e-2 gate) which halves HBM input traffic — the
    bottleneck (memory-bound problem). Output is stored bf16 too and
    upcast on the host. Bonus: bf16 matmuls run at 1 cycle/row on the PE
    vs fp32r's 4 at peak clock, so TensorE drops out of the picture.
    Same schedule skeleton as the fp32r variant."""
    from concourse import bacc, tile, mybir

    nc = bacc.Bacc("TRN2", target_bir_lowering=False, debug=False, enable_asserts=False)
    dt = mybir.dt.float32
    mdt = mybir.dt.bfloat16
    odt = mybir.dt.bfloat16
    SG = 8                      # patches per PSUM super-group
    NSG = PL // SG              # 16
    xf = nc.dram_tensor("xf", [KR, PL, NQ, N + FOUT], mdt, kind="ExternalInput").ap()
    bt = nc.dram_tensor("bt", [FOUT, 1], dt, kind="ExternalInput").ap()
    out = nc.dram_tensor("out", [FOUT, PL, N], odt, kind="ExternalOutput").ap()

    sizes = list(sizes)
    assert sum(sizes) == PL
    groups = [(g * SG, SG) for g in range(NSG - 1)] + [(PL - 8, 4), (PL - 4, 4)]
    gof = {}
    for gi, (s0, gsz) in enumerate(groups):
        for i in range(gsz):
            gof[s0 + i] = (gi, i)
    relu = mybir.ActivationFunctionType.Relu

    with tile.TileContext(nc) as tc:
        with (
            tc.tile_pool(name="xfpool", bufs=bufs) as xfpool,
            tc.tile_pool(name="psum", bufs=6, space="PSUM") as psum,
            tc.tile_pool(name="misc", bufs=1) as misc,
        ):
            # bias rides the scalar ring (a tiny DMA at the head of a ring
            # stalls that ring ~3us before its next bulk transfer — so it
            # must NOT share a ring with the input stream).
            bias_t = misc.tile([FOUT, 1], dt)
            nc.scalar.dma_start(bias_t[:], bt[:])
            staging = misc.tile([FOUT, PL, N], odt)

            # All input loads on sync's single HWDGE ring, issued upfront:
            # strictly in-order completions at ~390 GB/s. Few, large chunks
            # (each ring DMA costs ~1us of ring stall at its completion
            # boundary, and small chunks make the per-chunk engine stagger
            # dominate); one tiny final chunk keeps the last
            # data->matmul->ACT->store chain short.
            p0 = 0
            tiles = []
            for ch, PC in enumerate(sizes):
                xtile = xfpool.tile([KR, PC, NQ, N + FOUT], mdt, tag="xf")
                nc.sync.dma_start(xtile[:], xf[:, p0 : p0 + PC, :, :])
                tiles.append((xtile, p0, PC))
                p0 += PC

            ptile = None
            for ch, (xtile, p0, PC) in enumerate(tiles):
                for pl in range(PC):
                    p = p0 + pl
                    gi, i = gof[p]
                    s0, gsz = groups[gi]
                    if i == 0:
                        ptile = psum.tile([FOUT, SG, N], dt, tag="ps")
                    for q in range(NQ):
                        nc.tensor.matmul(
                            ptile[:, i, :],
                            xtile[:, pl, q, N : N + FOUT],  # lhsT [128,32(o)]
                            xtile[:, pl, q, 0:N],           # rhs  [128,32(b)]
                            start=(q == 0),
                            stop=(q == NQ - 1),
                        )
                    if i == gsz - 1:
                        if act == "scalar":
                            nc.scalar.activation(
                                staging[:, s0 : s0 + gsz, :],
                                ptile[:, :gsz, :],
                                relu,
                                bias=bias_t[:],
                            )
                        else:
                            # DVE: max(psum + bias, 0) in one instruction;
                            # no ACT table load, and the store issues on
                            # scalar no longer queue behind ACT execution.
                            nc.vector.tensor_scalar(
                                staging[:, s0 : s0 + gsz, :],
                                ptile[:, :gsz, :],
                                bias_t[:],
                                0.0,
                                mybir.AluOpType.add,
                                mybir.AluOpType.max,
                            )
                        # Stores ride the scalar ring. Bulk ones lag a few
                        # groups behind the ACT stream; near the end store
                        # eagerly so the final chain is just
                        # ACT -> one small store.
                        if gi == len(groups) - 1:
                            a = groups[gi - 1][0]
                            nc.scalar.dma_start(
                                out[:, a:PL, :], staging[:, a:PL, :]
                            )
                        elif gi == len(groups) - 3:
                            a = groups[7][0]
                            nc.scalar.dma_start(
                                out[:, a:s0 + gsz, :], staging[:, a:s0 + gsz, :]
                            )
                        elif gi == 7:
                            nc.scalar.dma_start(
                                out[:, 0:s0, :], staging[:, 0:s0, :]
                            )
    nc.compile()
    return nc


def _build_module_bf16_s(
    bulk_sizes=(32, 32, 32), tail_sizes=(16, 8, 8), bufs=3
):
    """Hybrid-precision variant. Patches 0..sum(bulk_sizes) carry the X
    operand of their last two K-chunks (q=2,3) as fp8-e4m3 (filters stay
    bf16), cutting input traffic 12.5% on the bulk at a measured total
    rel err of ~1.6e-2 (< the 2e-2 gate; pure bf16 is 2.9e-3). The tail
    patches stay fully bf16 with the proven single-DMA-per-chunk layout
    so the end-of-stream chain is unchanged.

    Bulk chunks load three tensors each (bf16 q01 pack, bf16 q23
    filters, fp8 q23 X); all loads ride sync's ring in order. Semaphore-
    pool reuse waits land on sync's own issue stream only (benign: the
    ring stays several chunks deep)."""
    from concourse import bacc, tile, mybir

    nc = bacc.Bacc("TRN2", target_bir_lowering=False, debug=False, enable_asserts=False)
    dt = mybir.dt.float32
    mdt = mybir.dt.bfloat16
    f8 = mybir.dt.float8e4
    odt = mybir.dt.bfloat16
    SG = 8
    NSG = PL // SG
    BP = sum(bulk_sizes)              # bulk patch count
    TP = sum(tail_sizes)
    assert BP + TP == PL
    # xf1 packs, per (patch, qq in {0,1}): X_qq (32 cols) | F_qq (32) |
    # F_{qq+2} (32) — one bf16 stream; the q23 X rides separately as fp8.
    xf1 = nc.dram_tensor(
        "xf1", [KR, BP, 2, N + 2 * FOUT], mdt, kind="ExternalInput"
    ).ap()
    xf2x = nc.dram_tensor("xf2x", [KR, BP, 2, N], f8, kind="ExternalInput").ap()
    xft = nc.dram_tensor("xft", [KR, TP, NQ, N + FOUT], mdt, kind="ExternalInput").ap()
    bt = nc.dram_tensor("bt", [FOUT, 1], dt, kind="ExternalInput").ap()
    out = nc.dram_tensor("out", [FOUT, PL, N], odt, kind="ExternalOutput").ap()

    groups = [(g * SG, SG) for g in range(NSG - 1)] + [(PL - 8, 4), (PL - 4, 4)]
    gof = {}
    for gi, (s0, gsz) in enumerate(groups):
        for i in range(gsz):
            gof[s0 + i] = (gi, i)
    relu = mybir.ActivationFunctionType.Relu

    with tile.TileContext(nc) as tc:
        with (
            tc.tile_pool(name="xfpool", bufs=bufs) as xfpool,
            tc.tile_pool(name="psum", bufs=6, space="PSUM") as psum,
            tc.tile_pool(name="misc", bufs=1) as misc,
        ):
            bias_t = misc.tile([FOUT, 1], dt)
            nc.scalar.dma_start(bias_t[:], bt[:])
            staging = misc.tile([FOUT, PL, N], odt)

            # chunk list: (kind, tiles, p0, PC); bulk then tail.
            # (Moving the fp8 side-stream to the scalar ring as one DMA
            # was tried and measured ~1.3us WORSE: the dual-ring packet
            # interleave slows the bulk chunks' completions more than the
            # small per-chunk fp8 DMAs cost on sync's ring.)
            chunks = []
            p0 = 0
            for PC in bulk_sizes:
                t1 = xfpool.tile([KR, PC, 2, N + 2 * FOUT], mdt, tag="x1")
                t2x = xfpool.tile([KR, PC, 2, N], f8, tag="x2x")
                nc.sync.dma_start(t1[:], xf1[:, p0 : p0 + PC, :, :])
                nc.sync.dma_start(t2x[:], xf2x[:, p0 : p0 + PC, :, :])
                chunks.append(("b", (t1, t2x), p0, PC))
                p0 += PC
            q0 = 0
            for PC in tail_sizes:
                tt = xfpool.tile([KR, PC, NQ, N + FOUT], mdt, tag="xt")
                nc.sync.dma_start(tt[:], xft[:, q0 : q0 + PC, :, :])
                chunks.append(("t", (tt,), BP + q0, PC))
                q0 += PC

            ptile = None
            for kind, tiles, p0, PC in chunks:
                for pl in range(PC):
                    p = p0 + pl
                    gi, i = gof[p]
                    s0, gsz = groups[gi]
                    if i == 0:
                        ptile = psum.tile([FOUT, SG, N], dt, tag="ps")
                    for q in range(NQ):
                        if kind == "t":
                            tt = tiles[0]
                            lhsT = tt[:, pl, q, N : N + FOUT]
                            rhs = tt[:, pl, q, 0:N]
                        elif q < 2:
                            t1 = tiles[0]
                            lhsT = t1[:, pl, q, N : N + FOUT]
                            rhs = t1[:, pl, q, 0:N]
                        else:
                            lhsT = tiles[0][:, pl, q - 2, N + FOUT : N + 2 * FOUT]
                            rhs = tiles[1][:, pl, q - 2, :]
                        nc.tensor.matmul(
                            ptile[:, i, :],
                            lhsT,
                            rhs,
                            start=(q == 0),
                            stop=(q == NQ - 1),
                        )
                    if i == gsz - 1:
                        nc.scalar.activation(
                            staging[:, s0 : s0 + gsz, :],
                            ptile[:, :gsz, :],
                            relu,
                            bias=bias_t[:],
                        )
                        if gi == len(groups) - 1:
                            a = groups[gi - 1][0]
                            nc.scalar.dma_start(
                                out[:, a:PL, :], staging[:, a:PL, :]
                            )
                        elif gi == len(groups) - 3:
                            a = groups[7][0]
                            nc.scalar.dma_start(
                                out[:, a : s0 + gsz, :],
                                staging[:, a : s0 + gsz, :],
                            )
                        elif gi == 7:
                            nc.scalar.dma_start(
                                out[:, 0:s0, :], staging[:, 0:s0, :]
                            )
    nc.compile()
    return nc


def _build_module_bf16_t(a_sizes=(32, 32, 32, 16), b_size=16, bufs=5):
    """bf16 with the tail patches carried by the scalar ring, issued
    upfront: their data lands mid-stream, so the PE's final wait is only
    for the LAST sync-ring chunk; the B patches fill the completion-lag
    bubble before it. Processing order: A chunks ..., B, last A chunk."""
    from concourse import bacc, tile, mybir

    nc = bacc.Bacc("TRN2", target_bir_lowering=False, debug=False, enable_asserts=False)
    dt = mybir.dt.float32
    mdt = mybir.dt.bfloat16
    odt = mybir.dt.bfloat16
    SG = 8
    NSG = PL // SG
    xf = nc.dram_tensor("xf", [KR, PL, NQ, N + FOUT], mdt, kind="ExternalInput").ap()
    bt = nc.dram_tensor("bt", [FOUT, 1], dt, kind="ExternalInput").ap()
    out = nc.dram_tensor("out", [FOUT, PL, N], odt, kind="ExternalOutput").ap()

    a_sizes = list(a_sizes)
    assert sum(a_sizes) + b_size == PL
    groups = [(g * SG, SG) for g in range(NSG - 1)] + [(PL - 8, 4), (PL - 4, 4)]
    gof = {}
    for gi, (s0, gsz) in enumerate(groups):
        for i in range(gsz):
            gof[s0 + i] = (gi, i)
    relu = mybir.ActivationFunctionType.Relu

    a_edge = PL - b_size          # start of B's patch range
    # chunks in EMISSION order for loads; processing order reorders below
    with tile.TileContext(nc) as tc:
        with (
            tc.tile_pool(name="xfpool", bufs=bufs) as xfpool,
            tc.tile_pool(name="psum", bufs=6, space="PSUM") as psum,
            tc.tile_pool(name="misc", bufs=1) as misc,
        ):
            staging = misc.tile([FOUT, PL, N], odt)
            bias_t = misc.tile([FOUT, 1], dt)

            # loads: A chunks on sync upfront; B chunk on scalar FIRST
            # (a tiny DMA at a ring head stalls the ring ~3us, so bias
            # rides scalar AFTER the bulk B chunk).
            chunks = []    # (xtile, p0, PC) keyed by patch range
            p0 = 0
            for PC in a_sizes:
                xtile = xfpool.tile([KR, PC, NQ, N + FOUT], mdt, tag="xf")
                nc.sync.dma_start(xtile[:], xf[:, p0 : p0 + PC, :, :])
                chunks.append((xtile, p0, PC))
                p0 += PC
            btile = xfpool.tile([KR, b_size, NQ, N + FOUT], mdt, tag="xf")
            nc.scalar.dma_start(btile[:], xf[:, a_edge:PL, :, :])
            nc.scalar.dma_start(bias_t[:], bt[:])

            # processing order: all A chunks but the last, then B, then
            # the last A chunk
            order = chunks[:-1] + [(btile, a_edge, b_size)] + [chunks[-1]]
            last_gi = gof[chunks[-1][1] + chunks[-1][2] - 1][0]

            ptile = None
            for xtile, p0, PC in order:
                for pl in range(PC):
                    p = p0 + pl
                    gi, i = gof[p]
                    s0, gsz = groups[gi]
                    if i == 0:
                        ptile = psum.tile([FOUT, SG, N], dt, tag="ps")
                    for q in range(NQ):
                        nc.tensor.matmul(
                            ptile[:, i, :],
                            xtile[:, pl, q, N : N + FOUT],
                            xtile[:, pl, q, 0:N],
                            start=(q == 0),
                            stop=(q == NQ - 1),
                        )
                    if i == gsz - 1:
                        nc.scalar.activation(
                            staging[:, s0 : s0 + gsz, :],
                            ptile[:, :gsz, :],
                            relu,
                            bias=bias_t[:],
                        )
                        if gi == 11:
                            nc.scalar.dma_start(
                                out[:, 0:96, :], staging[:, 0:96, :]
                            )
                        elif gi == len(groups) - 1:
                            nc.scalar.dma_start(
                                out[:, a_edge:PL, :], staging[:, a_edge:PL, :]
                            )
                        elif gi == last_gi:
                            nc.scalar.dma_start(
                                out[:, 96:a_edge, :], staging[:, 96:a_edge, :]
                            )
    nc.compile()
    return nc


def _get_module():
    if "nc" not in _CACHE:
        _CACHE["nc"] = _build_module()
    return _CACHE["nc"]


def _marshal(X, filters, bias, mdtype=np.float32):
    """Shard + lay out full inputs into per-core device arrays."""
    X = np.ascontiguousarray(np.asarray(X, dtype=np.float32))
    filters = np.ascontiguousarray(np.asarray(filters, dtype=np.float32))
    bias = np.asarray(bias, dtype=np.float32)

    # X: (b, core, pr, i, pc, j, c) -> (core, j, c, pr, pc, i, b)
    xv = X.reshape(N, NCORES, 4, FH, 32, FW, C)
    xt = xv.transpose(1, 5, 6, 2, 4, 3, 0).reshape(NCORES, KR, PL, NQ, N)
    # filters: (core, p, i, j, c, o) -> (core, j, c, p, i, o)
    fv = filters.reshape(NCORES, PL, FH, FW, C, FOUT)
    ft = fv.transpose(0, 3, 4, 1, 2, 5).reshape(NCORES, KR, PL, NQ, FOUT)
    xfa = np.concatenate([xt, ft], axis=4)
    if mdtype != np.float32:
        xfa = xfa.astype(mdtype)
    xfa = np.ascontiguousarray(xfa)
    bt = np.ascontiguousarray(np.tile(bias, 4).reshape(KR, 1))
    return xfa, bt


def _assemble(outs):
    """Per-core out [128=(s,o), NG, N] -> full (N, 32, 32, FOUT)."""
    z = np.stack(outs)                                  # (core, (s,o), g, b)
    z = z.reshape(NCORES, 4, FOUT, NG, N)               # (core, s, o, g, b)
    z = z.transpose(4, 0, 3, 1, 2)                      # (b, core, g, s, o)
    z = z.reshape(N, NCORES, PL, FOUT)                  # p_loc = 4*g + s
    z = z.reshape(N, NCORES * 4, 32, FOUT)              # (b, pr_glob, pc, o)
    return np.ascontiguousarray(z)


S_BULK = 96          # patches with q23-X in fp8 (see _build_module_bf16_s)


def _marshal_s(X, filters, bias):
    X = np.ascontiguousarray(np.asarray(X, dtype=np.float32))
    filters = np.ascontiguousarray(np.asarray(filters, dtype=np.float32))
    bias = np.asarray(bias, dtype=np.float32)
    xv = X.reshape(N, NCORES, 4, FH, 32, FW, C)
    xt = xv.transpose(1, 5, 6, 2, 4, 3, 0).reshape(NCORES, KR, PL, NQ, N)
    fv = filters.reshape(NCORES, PL, FH, FW, C, FOUT)
    ft = fv.transpose(0, 3, 4, 1, 2, 5).reshape(NCORES, KR, PL, NQ, FOUT)
    BP = S_BULK
    bf16 = ml_dtypes.bfloat16
    f8 = ml_dtypes.float8_e4m3fn
    xf1 = np.ascontiguousarray(
        np.concatenate(
            [xt[:, :, :BP, :2, :], ft[:, :, :BP, :2, :], ft[:, :, :BP, 2:, :]],
            axis=4,
        ).astype(bf16)
    )
    xf2x = np.ascontiguousarray(xt[:, :, :BP, 2:, :].astype(f8))
    xft = np.ascontiguousarray(
        np.concatenate([xt[:, :, BP:, :, :], ft[:, :, BP:, :, :]], axis=4)
        .astype(bf16)
    )
    bt = np.ascontiguousarray(bias.reshape(FOUT, 1))
    return xf1, xf2x, xft, bt


def _assemble_r(outs):
    """Per-core out [FOUT, PL, N] -> full (N, 32, 32, FOUT)."""
    z = np.stack(outs)                                  # (core, o, p, b)
    z = z.transpose(3, 0, 2, 1)                         # (b, core, p, o)
    return np.ascontiguousarray(z.reshape(N, 32, 32, FOUT))


LAST_RESULT = None
VARIANT = "bf16s"


def kernel(X, filters, bias):
    global LAST_RESULT
    from concourse import bass_utils
    from concourse.bass_utils import run_bass_kernel_spmd

    # If tracing is enabled in the environment, keep the artifact upload
    # local so a missing bucket can't fail the run.
    bass_utils.upload_artifacts = lambda tmpdir: f"local://{tmpdir}"

    if "nc" not in _CACHE:
        _CACHE["nc"] = {
            "fp32r": _build_module_r,
            "bf16": _build_module_bf16,
            "bf16t": _build_module_bf16_t,
            "bf16s": _build_module_bf16_s,
            "fp32": _build_module,
        }[VARIANT]()
    nc = _CACHE["nc"]
    if VARIANT == "bf16s":
        xf1, xf2x, xft, bt = _marshal_s(X, filters, bias)
        in_maps = [
            {"xf1": xf1[k], "xf2x": xf2x[k], "xft": xft[k], "bt": bt}
            for k in range(NCORES)
        ]
    else:
        mdtype = ml_dtypes.bfloat16 if VARIANT.startswith("bf16") else np.float32
        xfa, bt = _marshal(X, filters, bias, mdtype=mdtype)
        if VARIANT != "fp32":
            bt = np.ascontiguousarray(bt[:FOUT])
        in_maps = [{"xf": xfa[k], "bt": bt} for k in range(NCORES)]
    res = run_bass_kernel_spmd(nc, in_maps, core_ids=list(range(NCORES)))
    LAST_RESULT = res
    outs = [res.results[k]["out"] for k in range(NCORES)]
    if VARIANT == "fp32":
        return _assemble(outs)
    z = _assemble_r(outs)
    return np.ascontiguousarray(z.astype(np.float32)) if z.dtype != np.float32 else z

